# revision 14
# baseline (speedup 1.0000x reference)
"""KAN-FNO block on 8 Trainium2 NeuronCores — hand-written Bass/Tile kernel.

v2: data-parallel over batch (16 -> 2 per core), weights replicated.
Spectral conv = truncated DFT via PE matmuls (as v1) with consolidated
PSUM->SBUF copies round-robined over Scalar/Vector/GpSimd.  KAN layers:
conv bias + z-branch accumulated in PSUM via PE (dup'd weight columns give
a [128,1024] psum holding y twice), basis = 4 ACT |.| ops (per-partition
pair shifts) + 1 stock tensor_scalar min (bf16 2x) + 1 fused DVE cube op
f = 4n^3 - m^3; silu via Tanh ACT + GpSimd scalar_tensor_tensor; exact
Gelu.  Every ACT function lives in the gelu_and_others table set -> no
ACT_TABLE_LOADs inside the loop.
"""
import numpy as np
import ml_dtypes

import concourse.bass as bass
import concourse.bacc as bacc
import concourse.mybir as mybir
from concourse.tile import TileContext
from concourse import bass_utils
from concourse import dve_ops
from concourse.dve_spec import (Spec, Src0, C0, One, Zero, minn, sq, lower,
                                _has_src1)
from concourse.dve_uop import DveOpSpec

BF16 = mybir.dt.bfloat16
F32 = mybir.dt.float32

B, C, H, W = 16, 64, 128, 128
NCORES = 8
BLOC = B // NCORES          # 2 samples per core
MODES = 16                  # kept w-freqs
R32 = 32                    # kept h-freqs (16 low + 16 high)
HKNOT = 0.4
GRID0 = -2.2                # first knot
PIX = H * W                 # 16384 pixels per sample
CHUNK = 1024                # KAN pixel chunk (8 h-rows)
NCH = PIX // CHUNK          # chunks per sample
HROWS = CHUNK // W          # h-rows per chunk

# ---------------------------------------------------------------- custom DVE
_REGISTERED = {}


def _register_dve():
    if _REGISTERED:
        return _REGISTERED

    def mk(name, spec):
        row = max(dve_ops._SUB_OPCODE_FOR_NAME.values()) + 1
        assert row < 0x20
        dve_ops._SUB_OPCODE_FOR_NAME[name] = row
        op = dve_ops.DveOp(name, spec, subdim=False, uops_sha={})
        for ver in ("v3", "v4"):
            try:
                r = DveOpSpec(name=name, opcode=row, uops=lower(spec, ver=ver),
                              rd1_en=_has_src1(spec))
                op.uops_sha[ver] = r.sha(ver)
            except Exception:
                pass
        dve_ops.OPS.append(op)
        dve_ops.CUSTOM_DVE_SPECS[name] = spec
        return op

    # in0 = m = min(a-2, 0) = -relu(2-a) with a = |u-2|.
    # n = min(m+1, 0) = -relu(1-a);  out = s0*n^3 - m^3 = relu(2-a)^3
    # - 4*relu(1-a)^3 = 6*B(u-2) for s0=4.
    _n = minn(Src0 + One, Zero)
    cube = Spec(
        body=C0 * (_n * sq(_n)) - Src0 * sq(Src0),
        reference=lambda in0, in1, s0, s1, imm2:
            (s0 * np.minimum(in0 + 1.0, 0.0) ** 3 - in0 ** 3
             ).astype(np.float32),
    )
    _REGISTERED['cube'] = mk("KANFNO_CUBE3", cube)
    return _REGISTERED


# ---------------------------------------------------------------- host consts
def _np_consts():
    r_idx = np.concatenate([np.arange(MODES), np.arange(H - MODES, H)])
    h = np.arange(H)
    th = 2 * np.pi * np.outer(r_idx, h) / H            # (32,128)
    AhT = np.zeros((H, 64), np.float32)
    AhT[:, :32] = np.cos(th).T
    AhT[:, 32:] = np.sin(th).T
    w = np.arange(W)
    k = np.arange(MODES)
    ph = 2 * np.pi * np.outer(k, w) / W                # (16,128)
    Fw = np.zeros((W, 32), np.float32)
    Fw[:, :16] = np.cos(ph).T
    Fw[:, 16:] = np.sin(ph).T
    gam = np.full(MODES, 2.0 / (H * W))
    gam[0] = 1.0 / (H * W)
    G1 = (np.cos(ph) * gam[:, None]).astype(np.float32)   # (16,128)
    G2 = (np.sin(ph) * gam[:, None]).astype(np.float32)
    AhI = np.zeros((64, H), np.float32)
    AhI[:32] = np.cos(th)
    AhI[32:] = -np.sin(th)
    # ACT |.| bias for 4 dual pairs: rows 0:64 -> basis 2p, 64:128 -> 2p+1
    biasv = np.zeros((128, 4), np.float32)
    for p in range(4):
        for half in range(2):
            j = 2 * p + half
            gj = GRID0 + HKNOT * j
            biasv[half * 64:(half + 1) * 64, p] = -(gj / HKNOT + 2.0)
    ident = np.eye(128, dtype=np.float32)
    return AhT, Fw, G1, G2, AhI, biasv, ident


def _bf16(a):
    return np.asarray(a, dtype=ml_dtypes.bfloat16)


def _host_prep(x, spec_w1_r, spec_w1_i, spec_w2_r, spec_w2_i, conv_w, conv_b,
               k1_base, k1_spline, k1_scaler, k2_base, k2_spline, k2_scaler):
    """Build the per-core input maps (core-invariant weights + per-core x)."""
    AhT, Fw, G1, G2, AhI, biasv, ident = _np_consts()
    # mix blocks -> (32, 128, 16, 128) [r, kappa, k, m] for clean DMA
    Wr = np.concatenate([spec_w1_r, spec_w2_r], axis=2)  # (i,o,32,16)
    Wi = np.concatenate([spec_w1_i, spec_w2_i], axis=2)
    blk = np.empty((R32, 128, MODES, 128), np.float32)
    blk[:, :64, :, :64] = Wr.transpose(2, 0, 3, 1)       # (r,i,k,o)
    blk[:, 64:, :, :64] = Wi.transpose(2, 0, 3, 1)
    blk[:, :64, :, 64:] = Wi.transpose(2, 0, 3, 1)
    blk[:, 64:, :, 64:] = -Wr.transpose(2, 0, 3, 1)

    def kan_w(base_w, spline_w, scaler, dup):
        Wm = (spline_w * scaler[..., None]) / 6.0        # (o, i, 8)
        wk = np.empty((4, 128, 64), np.float32)
        for kc in range(4):
            wk[kc, :64] = Wm[:, :, 2 * kc].T             # (i, o)
            wk[kc, 64:] = Wm[:, :, 2 * kc + 1].T
        base = 0.5 * base_w.T                            # silu = x(1+t)/2
        if dup:
            wk = np.concatenate([wk, wk], axis=2)        # (4, 128, 128)
            base = np.concatenate([base, base], axis=1)  # (64, 128)
        # flatten kan spline stationaries side by side
        wflat = np.concatenate([wk[kc] for kc in range(4)], axis=1)
        return wflat, np.ascontiguousarray(base)

    k1w, k1b = kan_w(k1_base, k1_spline, k1_scaler, True)
    k2w, k2b = kan_w(k2_base, k2_spline, k2_scaler, False)

    # conv with output-duplicated columns + bias row (ones-row trick)
    cw = np.concatenate([conv_w.T, conv_w.T], axis=1)    # (64, 128)
    convw_aug = np.concatenate(
        [cw, np.concatenate([conv_b, conv_b])[None, :]], axis=0)  # (65, 128)

    shared = {
        'aht': _bf16(AhT), 'fw': _bf16(Fw),
        'g1': _bf16(G1), 'g2': _bf16(G2), 'ahi': _bf16(AhI),
        'wblk': _bf16(blk.reshape(R32, 128, MODES * 128)),
        'convw_aug': _bf16(convw_aug),
        'k1w': _bf16(k1w), 'k1b': _bf16(k1b),
        'k2w': _bf16(k2w), 'k2b': _bf16(k2b),
        'biasv': biasv.astype(np.float32), 'ident': _bf16(ident),
    }
    xb = _bf16(x).reshape(NCORES, BLOC * C, H, W)
    in_maps = []
    for core in range(NCORES):
        m = dict(shared)
        m['x'] = np.ascontiguousarray(xb[core])
        in_maps.append(m)
    return in_maps


# ---------------------------------------------------------------- bass build
def build_nc():
    ops = _register_dve()
    nc = bacc.Bacc("TRN2", target_bir_lowering=False, debug=False)
    ALU = mybir.AluOpType

    x_in = nc.dram_tensor("x", [BLOC * C, H, W], BF16, kind="ExternalInput")
    aht = nc.dram_tensor("aht", [H, 64], BF16, kind="ExternalInput")
    fw = nc.dram_tensor("fw", [W, 32], BF16, kind="ExternalInput")
    g1 = nc.dram_tensor("g1", [MODES, W], BF16, kind="ExternalInput")
    g2 = nc.dram_tensor("g2", [MODES, W], BF16, kind="ExternalInput")
    ahi = nc.dram_tensor("ahi", [64, H], BF16, kind="ExternalInput")
    wblk = nc.dram_tensor("wblk", [R32, 128, MODES * 128], BF16,
                          kind="ExternalInput")
    convw_aug = nc.dram_tensor("convw_aug", [65, 128], BF16,
                               kind="ExternalInput")
    k1w = nc.dram_tensor("k1w", [128, 512], BF16, kind="ExternalInput")
    k1b = nc.dram_tensor("k1b", [64, 128], BF16, kind="ExternalInput")
    k2w = nc.dram_tensor("k2w", [128, 256], BF16, kind="ExternalInput")
    k2b = nc.dram_tensor("k2b", [64, 64], BF16, kind="ExternalInput")
    biasv = nc.dram_tensor("biasv", [128, 4], F32, kind="ExternalInput")
    ident = nc.dram_tensor("ident", [128, 128], BF16, kind="ExternalInput")
    out = nc.dram_tensor("out", [BLOC * C, PIX], BF16, kind="ExternalOutput")

    AF = mybir.ActivationFunctionType

    # round-robin PSUM->SBUF copies across Scalar/Vector/GpSimd
    cp_n = [0]

    def cp(dst, src):
        k = cp_n[0] % 2
        cp_n[0] += 1
        if k == 0:
            nc.scalar.activation(dst, src, AF.Copy)
        else:
            nc.vector.tensor_copy(dst, src)

    with TileContext(nc) as tc:
        with tc.tile_pool(name="consts", bufs=1) as pconst, \
                tc.tile_pool(name="zs_pool", bufs=1) as pzs:
            aht_s = pconst.tile([H, 64], BF16)
            nc.sync.dma_start(aht_s[:], aht[:, :])
            fw_s = pconst.tile([W, 32], BF16)
            nc.sync.dma_start(fw_s[:], fw[:, :])
            g1_s = pconst.tile([MODES, W], BF16)
            nc.sync.dma_start(g1_s[:], g1[:, :])
            g2_s = pconst.tile([MODES, W], BF16)
            nc.sync.dma_start(g2_s[:], g2[:, :])
            ahi_s = pconst.tile([64, H], BF16)
            nc.sync.dma_start(ahi_s[:], ahi[:, :])
            convw_s = pconst.tile([65, 128], BF16)
            nc.sync.dma_start(convw_s[:], convw_aug[:, :])
            k1w_s = pconst.tile([128, 512], BF16)
            nc.sync.dma_start(k1w_s[:], k1w[:, :])
            k1b_s = pconst.tile([64, 128], BF16)
            nc.sync.dma_start(k1b_s[:], k1b[:, :])
            k2w_s = pconst.tile([128, 256], BF16)
            nc.sync.dma_start(k2w_s[:], k2w[:, :])
            k2b_s = pconst.tile([64, 64], BF16)
            nc.sync.dma_start(k2b_s[:], k2b[:, :])
            biasv_s = pconst.tile([128, 4], F32)
            nc.sync.dma_start(biasv_s[:], biasv[:, :])
            ident_s = pconst.tile([128, 128], BF16)
            nc.sync.dma_start(ident_s[:], ident[:, :])

            zs = [pzs.tile([128, C * H], BF16, name=f"zs{b_}")
                  for b_ in range(BLOC)]

            # ================= spectral branch =================
            with tc.tile_pool(name="sb1", bufs=1) as ps1:
                # ---- stage A (h-DFT): t1[w, bc*64+rho] ----
                t1 = ps1.tile([W, BLOC * C * 64], BF16, name="t1")
                with tc.tile_pool(name="pxa", bufs=2) as pxa:
                    for b_ in range(BLOC):
                        xa = pxa.tile([H, C * W], BF16, tag="xa")
                        nc.sync.dma_start(
                            xa[:].rearrange("h (c w) -> h c w", w=W),
                            x_in.ap().rearrange("(b c) h w -> h b c w",
                                                b=BLOC)[:, b_, :, :])
                        with tc.tile_pool(name="pA", bufs=2,
                                          space="PSUM") as pa:
                            for cg in range(8):   # 8 channels per psum tile
                                p1 = pa.tile([W, 512], F32, tag="p1")
                                for ci in range(8):
                                    c_ = cg * 8 + ci
                                    nc.tensor.matmul(
                                        p1[:, ci * 64:(ci + 1) * 64],
                                        xa[:, c_ * W:(c_ + 1) * W],
                                        aht_s[:], start=True, stop=True)
                                cp(t1[:, (b_ * C + cg * 8) * 64:
                                      (b_ * C + cg * 8 + 8) * 64], p1[:])

                # ---- stage B (w-DFT) + quadrant combine ----
                qr = [ps1.tile([16, 2048], BF16, name=f"qr{b_}")
                      for b_ in range(BLOC)]
                qi = [ps1.tile([16, 2048], BF16, name=f"qi{b_}")
                      for b_ in range(BLOC)]
                with tc.tile_pool(name="pB", bufs=1, space="PSUM") as pb:
                    # p2s: rows b0@0, b1@32; free [kc block 0:4096 | ks 4096:]
                    p2s = ps1.tile([64, 8192], BF16, name="p2s")
                    for half, fsl in enumerate((slice(0, 16), slice(16, 32))):
                        p2 = pb.tile([64, 4096], F32, tag="p2")
                        for bc in range(BLOC * C):
                            b_, c_ = divmod(bc, C)
                            nc.tensor.matmul(
                                p2[32 * b_:32 * b_ + 16,
                                   64 * c_:64 * c_ + 64],
                                fw_s[:, fsl], t1[:, bc * 64:(bc + 1) * 64],
                                start=True, stop=True)
                        for b_ in range(BLOC):
                            for hh in range(2):
                                cp(p2s[32 * b_:32 * b_ + 16,
                                       4096 * half + 2048 * hh:
                                       4096 * half + 2048 * (hh + 1)],
                                   p2[32 * b_:32 * b_ + 16,
                                      2048 * hh:2048 * (hh + 1)])
                    p2v = p2s[:].rearrange("p (h c r) -> p h c r", h=2, c=64)
                    for b_ in range(BLOC):
                        o_r = qr[b_][:].rearrange("p (r c) -> p c r", c=64)
                        o_i = qi[b_][:].rearrange("p (r c) -> p c r", c=64)
                        sl = slice(32 * b_, 32 * b_ + 16)
                        nc.vector.tensor_sub(
                            o_r, p2v[sl, 0, :, 0:32], p2v[sl, 1, :, 32:64])
                        nc.vector.tensor_add(
                            o_i, p2v[sl, 0, :, 32:64], p2v[sl, 1, :, 0:32])

                # ---- mix-input transposes -> M_r (4 r per psum tile) ----
                m4_tiles = []
                with tc.tile_pool(name="pM", bufs=2, space="PSUM") as pm:
                    for g in range(8):
                        mp = pm.tile([128, 128], BF16, tag="mp")
                        for j in range(4):
                            r = g * 4 + j
                            for b_ in range(BLOC):
                                col = j * 32 + b_ * MODES
                                nc.tensor.matmul(
                                    mp[0:64, col:col + MODES],
                                    qr[b_][:, r * 64:(r + 1) * 64],
                                    ident_s[0:16, 0:16], is_transpose=True,
                                    start=True, stop=True)
                                nc.tensor.matmul(
                                    mp[64:128, col:col + MODES],
                                    qi[b_][:, r * 64:(r + 1) * 64],
                                    ident_s[0:16, 0:16], is_transpose=True,
                                    start=True, stop=True)
                        ms = ps1.tile([128, 128], BF16, name=f"m4_{g}")
                        cp(ms[:], mp[:])
                        m4_tiles.append(ms)

                # ---- channel mix: ys cols j*32 + k*2 + b ----
                ys4 = []
                with tc.tile_pool(name="wbuf", bufs=6) as pw, \
                        tc.tile_pool(name="pY", bufs=2, space="PSUM") as py:
                    for g in range(8):
                        yp = py.tile([128, 128], F32, tag="yp")
                        for j in range(4):
                            r = g * 4 + j
                            wb = pw.tile([128, MODES * 128], BF16, tag="wb")
                            nc.sync.dma_start(wb[:], wblk[r, :, :])
                            mv = m4_tiles[g][:, j * 32:(j + 1) * 32]\
                                .rearrange("p (b k) -> p k b", k=MODES)
                            for kk in range(MODES):
                                nc.tensor.matmul(
                                    yp[:, j * 32 + kk * 2:j * 32 + kk * 2 + 2],
                                    wb[:, kk * 128:(kk + 1) * 128],
                                    mv[:, kk, :], start=True, stop=True)
                        ysr = ps1.tile([128, 128], BF16, name=f"ys4_{g}")
                        cp(ysr[:], yp[:])
                        ys4.append(ysr)

                # ---- Y transposes (8 pairs per tile, packed in free) ----
                yt_tiles = []
                with tc.tile_pool(name="pYT", bufs=2, space="PSUM") as pyt:
                    for g in range(8):
                        ytp = pyt.tile([16, 8 * 128], BF16, tag="ytp")
                        for s in range(8):
                            b_, r = divmod(g * 8 + s, R32)
                            ysv = ys4[r // 4][:, (r % 4) * 32:
                                              (r % 4) * 32 + 32]\
                                .rearrange("p (k b) -> p b k", b=BLOC)
                            nc.tensor.matmul(
                                ytp[:, s * 128:(s + 1) * 128],
                                ysv[:, b_, :], ident_s[:, :],
                                is_transpose=True, start=True, stop=True)
                        yts = ps1.tile([16, 8 * 128], BF16, name=f"yt{g}")
                        cp(yts[:], ytp[:])
                        yt_tiles.append(yts)

                # ---- I1 + combine -> Abr/Abi ----
                ab_r = [ps1.tile([128, 2048], BF16, name=f"abr{b_}")
                        for b_ in range(BLOC)]
                ab_i = [ps1.tile([128, 2048], BF16, name=f"abi{b_}")
                        for b_ in range(BLOC)]
                with tc.tile_pool(name="pI1", bufs=3, space="PSUM") as pi1, \
                        tc.tile_pool(name="pI1s", bufs=3) as pi1s:
                    for g in range(8):
                        for s in range(8):
                            b_, r = divmod(g * 8 + s, R32)
                            o1 = pi1.tile([128, 128], F32, tag="o1")
                            o2 = pi1.tile([128, 128], F32, tag="o2")
                            yts = yt_tiles[g][:, s * 128:(s + 1) * 128]
                            nc.tensor.matmul(o1[:], g1_s[:], yts,
                                             start=True, stop=True)
                            nc.tensor.matmul(o2[:], g2_s[:], yts,
                                             start=True, stop=True)
                            o1s = pi1s.tile([128, 128], BF16, tag="o1s")
                            cp(o1s[:], o1[:])
                            nc.vector.tensor_sub(
                                ab_r[b_][:, r * 64:(r + 1) * 64],
                                o1s[:, 0:64], o2[:, 64:128])
                            nc.vector.tensor_add(
                                ab_i[b_][:, r * 64:(r + 1) * 64],
                                o2[:, 0:64], o1s[:, 64:128])

                # ---- As transposes + I2 -> zs (2 e / 4 z per psum tile) ----
                with tc.tile_pool(name="pAS", bufs=2, space="PSUM") as pas, \
                        tc.tile_pool(name="as2p", bufs=2) as pas2, \
                        tc.tile_pool(name="pZ", bufs=2, space="PSUM") as pz:
                    pzt = None
                    for eg in range(32):
                        asp = pas.tile([64, 512], BF16, tag="asp")
                        for ei in range(2):
                            e = eg * 2 + ei
                            for half in range(2):
                                b_, o_ = divmod(e * 2 + half, C)
                                arv = ab_r[b_][:].rearrange(
                                    "p (r o) -> p o r", o=64)
                                aiv = ab_i[b_][:].rearrange(
                                    "p (r o) -> p o r", o=64)
                                coff = ei * 256 + half * 128
                                nc.tensor.matmul(
                                    asp[0:32, coff:coff + 128],
                                    arv[:, o_, :], ident_s[:, :],
                                    is_transpose=True, start=True, stop=True)
                                nc.tensor.matmul(
                                    asp[32:64, coff:coff + 128],
                                    aiv[:, o_, :], ident_s[:, :],
                                    is_transpose=True, start=True, stop=True)
                        as2 = pas2.tile([64, 512], BF16, tag="as2")
                        cp(as2[:], asp[:])
                        for ei in range(2):
                            for half in range(2):
                                mgl = eg * 4 + ei * 2 + half
                                b_, o_ = divmod(mgl, C)
                                slot = mgl % 4
                                if slot == 0:
                                    pzt = pz.tile([128, 512], F32, tag="zp")
                                nc.tensor.matmul(
                                    pzt[:, slot * 128:(slot + 1) * 128],
                                    as2[:, ei * 256 + half * 128:
                                        ei * 256 + half * 128 + 128],
                                    ahi_s[:], start=True, stop=True)
                                if slot == 3:
                                    cp(zs[b_][:, (o_ - 3) * H:(o_ + 1) * H],
                                       pzt[:])

            # ================= conv + z + KAN (software-pipelined) ====
            with tc.tile_pool(name="pk0", bufs=2, space="PSUM") as pk0, \
                    tc.tile_pool(name="pk1", bufs=1, space="PSUM") as pk1, \
                    tc.tile_pool(name="pk2", bufs=1, space="PSUM") as pk2, \
                    tc.tile_pool(name="pab", bufs=2) as pab, \
                    tc.tile_pool(name="pmb", bufs=2) as pmb, \
                    tc.tile_pool(name="pfb", bufs=3) as pfb, \
                    tc.tile_pool(name="psl", bufs=3) as psl, \
                    tc.tile_pool(name="pob", bufs=3) as pob, \
                    tc.tile_pool(name="pxt", bufs=4) as pxt:
                NCHT = BLOC * NCH
                st = [dict() for _ in range(NCHT)]

                def kan_s0(i):
                    b_, hg = divmod(i, NCH)
                    zv = zs[b_][:].rearrange("p (o h) -> p h o", o=64)
                    xt = pxt.tile([65, CHUNK], BF16, tag="xt")
                    nc.gpsimd.memset(xt[64:65, :], 1.0)
                    nc.sync.dma_start(
                        xt[0:64, :].rearrange("c (h w) -> c h w", w=W),
                        x_in.ap().rearrange("(b c) h w -> b c h w", b=BLOC)
                        [b_, :, hg * HROWS:(hg + 1) * HROWS, :])
                    P0 = pk0.tile([128, CHUNK], F32, tag="p0")
                    for n in range(2):
                        nc.tensor.matmul(
                            P0[:, n * 512:(n + 1) * 512], convw_s[:],
                            xt[:, n * 512:(n + 1) * 512],
                            start=True, stop=False)
                    for hh in range(HROWS):
                        h_ = hg * HROWS + hh
                        nc.tensor.matmul(
                            P0[0:64, hh * 128:(hh + 1) * 128],
                            zv[:, h_, :], ident_s[:, :],
                            start=False, stop=True)
                        nc.tensor.matmul(
                            P0[64:128, hh * 128:(hh + 1) * 128],
                            zv[:, h_, :], ident_s[:, :],
                            start=False, stop=True)
                    st[i]['P0'] = P0

                def elemwise(i, Pin, tagsfx):
                    """basis + silu staging from a [128,CHUNK] psum."""
                    ab_t = pab.tile([128, 4 * CHUNK], BF16, tag="ab" + tagsfx)
                    for p in range(4):
                        nc.scalar.activation(
                            ab_t[:, p * CHUNK:(p + 1) * CHUNK], Pin[:, :],
                            AF.Abs, bias=biasv_s[:, p:p + 1], scale=2.5)
                    tb = psl.tile([64, CHUNK], BF16, tag="tb" + tagsfx)
                    nc.scalar.activation(tb[:], Pin[0:64, :], AF.Tanh,
                                         scale=0.5)
                    mb = pmb.tile([128, 4 * CHUNK], BF16, tag="mb" + tagsfx)
                    nc.vector.tensor_scalar(mb[:], ab_t[:], 2.0, 0.0,
                                            ALU.subtract, ALU.min)
                    fb = pfb.tile([128, 4 * CHUNK], BF16, tag="fb" + tagsfx)
                    nc.vector._custom_dve(ops['cube'], out=fb[:], in0=mb[:],
                                          s0=4.0)
                    sl_t = psl.tile([64, CHUNK], BF16, tag="sl" + tagsfx)
                    nc.vector.scalar_tensor_tensor(
                        sl_t[:], tb[:], 1.0, Pin[0:64, :],
                        ALU.add, ALU.mult)
                    return fb, sl_t

                def kan_mm(Pout, rows, wsl, bsl, mwid, fb, sl_t):
                    for n in range(2):
                        sl = slice(n * 512, (n + 1) * 512)
                        for p in range(4):
                            nc.tensor.matmul(
                                Pout[rows, sl],
                                wsl[:, p * mwid:(p + 1) * mwid],
                                fb[:, p * CHUNK + n * 512:
                                   p * CHUNK + (n + 1) * 512],
                                start=(p == 0), stop=False)
                        nc.tensor.matmul(Pout[rows, sl], bsl[:],
                                         sl_t[:, sl], start=False, stop=True)

                def kan_s1(i):
                    st[i]['fb1'], st[i]['sl1'] = \
                        elemwise(i, st[i]['P0'], "1")

                def kan_s2(i):
                    P1 = pk1.tile([128, CHUNK], F32, tag="p1")
                    kan_mm(P1, slice(0, 128), k1w_s, k1b_s, 128,
                           st[i]['fb1'], st[i]['sl1'])
                    st[i]['fb2'], st[i]['sl2'] = elemwise(i, P1, "2")

                def kan_s3(i):
                    b_, hg = divmod(i, NCH)
                    P2 = pk2.tile([128, CHUNK], F32, tag="p2")
                    kan_mm(P2, slice(0, 64), k2w_s, k2b_s, 64,
                           st[i]['fb2'], st[i]['sl2'])
                    ob = pob.tile([64, CHUNK], BF16, tag="ob")
                    nc.scalar.activation(ob[:], P2[0:64, :], AF.Gelu)
                    nc.sync.dma_start(
                        out.ap()[b_ * C:(b_ + 1) * C,
                                 hg * CHUNK:(hg + 1) * CHUNK], ob[:])
                    st[i].clear()

                for i in range(NCHT + 3):
                    if i < NCHT:
                        kan_s0(i)
                    if 0 <= i - 1 < NCHT:
                        kan_s1(i - 1)
                    if 0 <= i - 3 < NCHT:
                        kan_s3(i - 3)
                    if 0 <= i - 2 < NCHT:
                        kan_s2(i - 2)
    nc.compile()
    return nc


# ---------------------------------------------------------------- runner
_NC = None


def _get_nc():
    global _NC
    if _NC is None:
        _NC = build_nc()
    return _NC


def kernel(x, spec_w1_r, spec_w1_i, spec_w2_r, spec_w2_i, conv_w, conv_b,
           k1_base, k1_spline, k1_scaler, k2_base, k2_spline, k2_scaler):
    nc = _get_nc()
    in_maps = _host_prep(x, spec_w1_r, spec_w1_i, spec_w2_r, spec_w2_i,
                         conv_w, conv_b, k1_base, k1_spline, k1_scaler,
                         k2_base, k2_spline, k2_scaler)
    res = bass_utils.run_bass_kernel_spmd(nc, in_maps,
                                          core_ids=list(range(NCORES)))
    outs = [res.results[c]['out'] for c in range(NCORES)]
    y = np.stack(outs).astype(np.float32)
    return y.reshape(B, C, H, W)


# revision 15
# speedup vs baseline: 1.0815x; 1.0815x over previous
"""KAN-FNO block on 8 Trainium2 NeuronCores — hand-written Bass/Tile kernel.

v2: data-parallel over batch (16 -> 2 per core), weights replicated.
Spectral conv = truncated DFT via PE matmuls (as v1) with consolidated
PSUM->SBUF copies round-robined over Scalar/Vector/GpSimd.  KAN layers:
conv bias + z-branch accumulated in PSUM via PE (dup'd weight columns give
a [128,1024] psum holding y twice), basis = 4 ACT |.| ops (per-partition
pair shifts) + 1 stock tensor_scalar min (bf16 2x) + 1 fused DVE cube op
f = 4n^3 - m^3; silu via Tanh ACT + GpSimd scalar_tensor_tensor; exact
Gelu.  Every ACT function lives in the gelu_and_others table set -> no
ACT_TABLE_LOADs inside the loop.
"""
import numpy as np
import ml_dtypes

import concourse.bass as bass
import concourse.bacc as bacc
import concourse.mybir as mybir
from concourse.tile import TileContext
from concourse import bass_utils
from concourse import dve_ops
from concourse.dve_spec import (Spec, Src0, C0, One, Zero, minn, sq, lower,
                                _has_src1)
from concourse.dve_uop import DveOpSpec

BF16 = mybir.dt.bfloat16
F32 = mybir.dt.float32

B, C, H, W = 16, 64, 128, 128
NCORES = 8
BLOC = B // NCORES          # 2 samples per core
MODES = 16                  # kept w-freqs
R32 = 32                    # kept h-freqs (16 low + 16 high)
HKNOT = 0.4
GRID0 = -2.2                # first knot
PIX = H * W                 # 16384 pixels per sample
CHUNK = 1024                # KAN pixel chunk (8 h-rows)
NCH = PIX // CHUNK          # chunks per sample
HROWS = CHUNK // W          # h-rows per chunk

# ---------------------------------------------------------------- custom DVE
_REGISTERED = {}


def _register_dve():
    if _REGISTERED:
        return _REGISTERED

    def mk(name, spec):
        row = max(dve_ops._SUB_OPCODE_FOR_NAME.values()) + 1
        assert row < 0x20
        dve_ops._SUB_OPCODE_FOR_NAME[name] = row
        op = dve_ops.DveOp(name, spec, subdim=False, uops_sha={})
        for ver in ("v3", "v4"):
            try:
                r = DveOpSpec(name=name, opcode=row, uops=lower(spec, ver=ver),
                              rd1_en=_has_src1(spec))
                op.uops_sha[ver] = r.sha(ver)
            except Exception:
                pass
        dve_ops.OPS.append(op)
        dve_ops.CUSTOM_DVE_SPECS[name] = spec
        return op

    # in0 = m = min(a-2, 0) = -relu(2-a) with a = |u-2|.
    # n = min(m+1, 0) = -relu(1-a);  out = s0*n^3 - m^3 = relu(2-a)^3
    # - 4*relu(1-a)^3 = 6*B(u-2) for s0=4.
    _n = minn(Src0 + One, Zero)
    cube = Spec(
        body=C0 * (_n * sq(_n)) - Src0 * sq(Src0),
        reference=lambda in0, in1, s0, s1, imm2:
            (s0 * np.minimum(in0 + 1.0, 0.0) ** 3 - in0 ** 3
             ).astype(np.float32),
    )
    _REGISTERED['cube'] = mk("KANFNO_CUBE3", cube)
    return _REGISTERED


# ---------------------------------------------------------------- host consts
def _np_consts():
    r_idx = np.concatenate([np.arange(MODES), np.arange(H - MODES, H)])
    h = np.arange(H)
    th = 2 * np.pi * np.outer(r_idx, h) / H            # (32,128)
    AhT = np.zeros((H, 64), np.float32)
    AhT[:, :32] = np.cos(th).T
    AhT[:, 32:] = np.sin(th).T
    w = np.arange(W)
    k = np.arange(MODES)
    ph = 2 * np.pi * np.outer(k, w) / W                # (16,128)
    Fw = np.zeros((W, 32), np.float32)
    Fw[:, :16] = np.cos(ph).T
    Fw[:, 16:] = np.sin(ph).T
    gam = np.full(MODES, 2.0 / (H * W))
    gam[0] = 1.0 / (H * W)
    G1 = (np.cos(ph) * gam[:, None]).astype(np.float32)   # (16,128)
    G2 = (np.sin(ph) * gam[:, None]).astype(np.float32)
    AhI = np.zeros((64, H), np.float32)
    AhI[:32] = np.cos(th)
    AhI[32:] = -np.sin(th)
    # ACT |.| bias for 4 dual pairs: rows 0:64 -> basis 2p, 64:128 -> 2p+1
    biasv = np.zeros((128, 4), np.float32)
    for p in range(4):
        for half in range(2):
            j = 2 * p + half
            gj = GRID0 + HKNOT * j
            biasv[half * 64:(half + 1) * 64, p] = -(gj / HKNOT + 2.0)
    ident = np.eye(128, dtype=np.float32)
    return AhT, Fw, G1, G2, AhI, biasv, ident


def _bf16(a):
    return np.asarray(a, dtype=ml_dtypes.bfloat16)


def _host_prep(x, spec_w1_r, spec_w1_i, spec_w2_r, spec_w2_i, conv_w, conv_b,
               k1_base, k1_spline, k1_scaler, k2_base, k2_spline, k2_scaler):
    """Build the per-core input maps (core-invariant weights + per-core x)."""
    AhT, Fw, G1, G2, AhI, biasv, ident = _np_consts()
    # mix blocks -> (32, 128, 16, 128) [r, kappa, k, m] for clean DMA
    Wr = np.concatenate([spec_w1_r, spec_w2_r], axis=2)  # (i,o,32,16)
    Wi = np.concatenate([spec_w1_i, spec_w2_i], axis=2)
    blk = np.empty((R32, 128, MODES, 128), np.float32)
    blk[:, :64, :, :64] = Wr.transpose(2, 0, 3, 1)       # (r,i,k,o)
    blk[:, 64:, :, :64] = Wi.transpose(2, 0, 3, 1)
    blk[:, :64, :, 64:] = Wi.transpose(2, 0, 3, 1)
    blk[:, 64:, :, 64:] = -Wr.transpose(2, 0, 3, 1)

    def kan_w(base_w, spline_w, scaler, dup):
        Wm = (spline_w * scaler[..., None]) / 6.0        # (o, i, 8)
        wk = np.empty((4, 128, 64), np.float32)
        for kc in range(4):
            wk[kc, :64] = Wm[:, :, 2 * kc].T             # (i, o)
            wk[kc, 64:] = Wm[:, :, 2 * kc + 1].T
        base = 0.5 * base_w.T                            # silu = x(1+t)/2
        if dup:
            wk = np.concatenate([wk, wk], axis=2)        # (4, 128, 128)
            base = np.concatenate([base, base], axis=1)  # (64, 128)
        # flatten kan spline stationaries side by side
        wflat = np.concatenate([wk[kc] for kc in range(4)], axis=1)
        return wflat, np.ascontiguousarray(base)

    k1w, k1b = kan_w(k1_base, k1_spline, k1_scaler, True)
    k2w, k2b = kan_w(k2_base, k2_spline, k2_scaler, False)

    # conv with output-duplicated columns + bias row (ones-row trick)
    cw = np.concatenate([conv_w.T, conv_w.T], axis=1)    # (64, 128)
    convw_aug = np.concatenate(
        [cw, np.concatenate([conv_b, conv_b])[None, :]], axis=0)  # (65, 128)

    shared = {
        'aht': _bf16(AhT), 'fw': _bf16(Fw),
        'g1': _bf16(G1), 'g2': _bf16(G2), 'ahi': _bf16(AhI),
        'wblk': _bf16(blk.reshape(R32, 128, MODES * 128)),
        'convw_aug': _bf16(convw_aug),
        'k1w': _bf16(k1w), 'k1b': _bf16(k1b),
        'k2w': _bf16(k2w), 'k2b': _bf16(k2b),
        'biasv': biasv.astype(np.float32), 'ident': _bf16(ident),
    }
    xb = _bf16(x).reshape(NCORES, BLOC * C, H, W)
    in_maps = []
    for core in range(NCORES):
        m = dict(shared)
        m['x'] = np.ascontiguousarray(xb[core])
        in_maps.append(m)
    return in_maps


# ---------------------------------------------------------------- bass build
def build_nc():
    ops = _register_dve()
    nc = bacc.Bacc("TRN2", target_bir_lowering=False, debug=False)
    ALU = mybir.AluOpType

    x_in = nc.dram_tensor("x", [BLOC * C, H, W], BF16, kind="ExternalInput")
    aht = nc.dram_tensor("aht", [H, 64], BF16, kind="ExternalInput")
    fw = nc.dram_tensor("fw", [W, 32], BF16, kind="ExternalInput")
    g1 = nc.dram_tensor("g1", [MODES, W], BF16, kind="ExternalInput")
    g2 = nc.dram_tensor("g2", [MODES, W], BF16, kind="ExternalInput")
    ahi = nc.dram_tensor("ahi", [64, H], BF16, kind="ExternalInput")
    wblk = nc.dram_tensor("wblk", [R32, 128, MODES * 128], BF16,
                          kind="ExternalInput")
    convw_aug = nc.dram_tensor("convw_aug", [65, 128], BF16,
                               kind="ExternalInput")
    k1w = nc.dram_tensor("k1w", [128, 512], BF16, kind="ExternalInput")
    k1b = nc.dram_tensor("k1b", [64, 128], BF16, kind="ExternalInput")
    k2w = nc.dram_tensor("k2w", [128, 256], BF16, kind="ExternalInput")
    k2b = nc.dram_tensor("k2b", [64, 64], BF16, kind="ExternalInput")
    biasv = nc.dram_tensor("biasv", [128, 4], F32, kind="ExternalInput")
    ident = nc.dram_tensor("ident", [128, 128], BF16, kind="ExternalInput")
    out = nc.dram_tensor("out", [BLOC * C, PIX], BF16, kind="ExternalOutput")

    AF = mybir.ActivationFunctionType

    # round-robin PSUM->SBUF copies across Scalar/Vector/GpSimd
    cp_n = [0]

    def cp(dst, src):
        k = cp_n[0] % 2
        cp_n[0] += 1
        if k == 0:
            nc.scalar.activation(dst, src, AF.Copy)
        else:
            nc.vector.tensor_copy(dst, src)

    with TileContext(nc) as tc:
        with tc.tile_pool(name="consts", bufs=1) as pconst, \
                tc.tile_pool(name="zs_pool", bufs=1) as pzs:
            aht_s = pconst.tile([H, 64], BF16)
            nc.sync.dma_start(aht_s[:], aht[:, :])
            fw_s = pconst.tile([W, 32], BF16)
            nc.sync.dma_start(fw_s[:], fw[:, :])
            g1_s = pconst.tile([MODES, W], BF16)
            nc.sync.dma_start(g1_s[:], g1[:, :])
            g2_s = pconst.tile([MODES, W], BF16)
            nc.sync.dma_start(g2_s[:], g2[:, :])
            ahi_s = pconst.tile([64, H], BF16)
            nc.sync.dma_start(ahi_s[:], ahi[:, :])
            convw_s = pconst.tile([65, 128], BF16)
            nc.sync.dma_start(convw_s[:], convw_aug[:, :])
            k1w_s = pconst.tile([128, 512], BF16)
            nc.sync.dma_start(k1w_s[:], k1w[:, :])
            k1b_s = pconst.tile([64, 128], BF16)
            nc.sync.dma_start(k1b_s[:], k1b[:, :])
            k2w_s = pconst.tile([128, 256], BF16)
            nc.sync.dma_start(k2w_s[:], k2w[:, :])
            k2b_s = pconst.tile([64, 64], BF16)
            nc.sync.dma_start(k2b_s[:], k2b[:, :])
            biasv_s = pconst.tile([128, 4], F32)
            nc.sync.dma_start(biasv_s[:], biasv[:, :])
            ident_s = pconst.tile([128, 128], BF16)
            nc.sync.dma_start(ident_s[:], ident[:, :])

            zs = [pzs.tile([128, C * H], BF16, name=f"zs{b_}")
                  for b_ in range(BLOC)]

            # ================= spectral branch =================
            with tc.tile_pool(name="sb1", bufs=1) as ps1:
                # ---- stage A (h-DFT): t1[w, bc*64+rho] ----
                t1 = ps1.tile([W, BLOC * C * 64], BF16, name="t1")
                with tc.tile_pool(name="pxa", bufs=2) as pxa:
                    for b_ in range(BLOC):
                        xa = pxa.tile([H, C * W], BF16, tag="xa")
                        nc.sync.dma_start(
                            xa[:].rearrange("h (c w) -> h c w", w=W),
                            x_in.ap().rearrange("(b c) h w -> h b c w",
                                                b=BLOC)[:, b_, :, :])
                        with tc.tile_pool(name="pA", bufs=2,
                                          space="PSUM") as pa:
                            for cg in range(8):   # 8 channels per psum tile
                                p1 = pa.tile([W, 512], F32, tag="p1")
                                for ci in range(8):
                                    c_ = cg * 8 + ci
                                    nc.tensor.matmul(
                                        p1[:, ci * 64:(ci + 1) * 64],
                                        xa[:, c_ * W:(c_ + 1) * W],
                                        aht_s[:], start=True, stop=True)
                                cp(t1[:, (b_ * C + cg * 8) * 64:
                                      (b_ * C + cg * 8 + 8) * 64], p1[:])

                # ---- stage B (w-DFT) + quadrant combine ----
                qr = [ps1.tile([16, 2048], BF16, name=f"qr{b_}")
                      for b_ in range(BLOC)]
                qi = [ps1.tile([16, 2048], BF16, name=f"qi{b_}")
                      for b_ in range(BLOC)]
                with tc.tile_pool(name="pB", bufs=1, space="PSUM") as pb:
                    # p2s: rows b0@0, b1@32; free [kc block 0:4096 | ks 4096:]
                    p2s = ps1.tile([64, 8192], BF16, name="p2s")
                    for half, fsl in enumerate((slice(0, 16), slice(16, 32))):
                        p2 = pb.tile([64, 4096], F32, tag="p2")
                        for bc in range(BLOC * C):
                            b_, c_ = divmod(bc, C)
                            nc.tensor.matmul(
                                p2[32 * b_:32 * b_ + 16,
                                   64 * c_:64 * c_ + 64],
                                fw_s[:, fsl], t1[:, bc * 64:(bc + 1) * 64],
                                start=True, stop=True)
                        for b_ in range(BLOC):
                            for hh in range(2):
                                cp(p2s[32 * b_:32 * b_ + 16,
                                       4096 * half + 2048 * hh:
                                       4096 * half + 2048 * (hh + 1)],
                                   p2[32 * b_:32 * b_ + 16,
                                      2048 * hh:2048 * (hh + 1)])
                    p2v = p2s[:].rearrange("p (h c r) -> p h c r", h=2, c=64)
                    for b_ in range(BLOC):
                        o_r = qr[b_][:].rearrange("p (r c) -> p c r", c=64)
                        o_i = qi[b_][:].rearrange("p (r c) -> p c r", c=64)
                        sl = slice(32 * b_, 32 * b_ + 16)
                        nc.vector.tensor_sub(
                            o_r, p2v[sl, 0, :, 0:32], p2v[sl, 1, :, 32:64])
                        nc.vector.tensor_add(
                            o_i, p2v[sl, 0, :, 32:64], p2v[sl, 1, :, 0:32])

                # ---- mix-input transposes -> M_r (4 r per psum tile) ----
                m4_tiles = []
                with tc.tile_pool(name="pM", bufs=2, space="PSUM") as pm:
                    for g in range(8):
                        mp = pm.tile([128, 128], BF16, tag="mp")
                        for j in range(4):
                            r = g * 4 + j
                            for b_ in range(BLOC):
                                col = j * 32 + b_ * MODES
                                nc.tensor.matmul(
                                    mp[0:64, col:col + MODES],
                                    qr[b_][:, r * 64:(r + 1) * 64],
                                    ident_s[0:16, 0:16], is_transpose=True,
                                    start=True, stop=True)
                                nc.tensor.matmul(
                                    mp[64:128, col:col + MODES],
                                    qi[b_][:, r * 64:(r + 1) * 64],
                                    ident_s[0:16, 0:16], is_transpose=True,
                                    start=True, stop=True)
                        ms = ps1.tile([128, 128], BF16, name=f"m4_{g}")
                        cp(ms[:], mp[:])
                        m4_tiles.append(ms)

                # ---- channel mix: ys cols j*32 + k*2 + b ----
                ys4 = []
                with tc.tile_pool(name="wbuf", bufs=6) as pw, \
                        tc.tile_pool(name="pY", bufs=2, space="PSUM") as py:
                    for g in range(8):
                        yp = py.tile([128, 128], F32, tag="yp")
                        for j in range(4):
                            r = g * 4 + j
                            wb = pw.tile([128, MODES * 128], BF16, tag="wb")
                            nc.sync.dma_start(wb[:], wblk[r, :, :])
                            mv = m4_tiles[g][:, j * 32:(j + 1) * 32]\
                                .rearrange("p (b k) -> p k b", k=MODES)
                            for kk in range(MODES):
                                nc.tensor.matmul(
                                    yp[:, j * 32 + kk * 2:j * 32 + kk * 2 + 2],
                                    wb[:, kk * 128:(kk + 1) * 128],
                                    mv[:, kk, :], start=True, stop=True)
                        ysr = ps1.tile([128, 128], BF16, name=f"ys4_{g}")
                        cp(ysr[:], yp[:])
                        ys4.append(ysr)

                # ---- Y transposes (8 pairs per tile, packed in free) ----
                yt_tiles = []
                with tc.tile_pool(name="pYT", bufs=2, space="PSUM") as pyt:
                    for g in range(8):
                        ytp = pyt.tile([16, 8 * 128], BF16, tag="ytp")
                        for s in range(8):
                            b_, r = divmod(g * 8 + s, R32)
                            ysv = ys4[r // 4][:, (r % 4) * 32:
                                              (r % 4) * 32 + 32]\
                                .rearrange("p (k b) -> p b k", b=BLOC)
                            nc.tensor.matmul(
                                ytp[:, s * 128:(s + 1) * 128],
                                ysv[:, b_, :], ident_s[:, :],
                                is_transpose=True, start=True, stop=True)
                        yts = ps1.tile([16, 8 * 128], BF16, name=f"yt{g}")
                        cp(yts[:], ytp[:])
                        yt_tiles.append(yts)

                # ---- I1 + combine -> Abr/Abi ----
                ab_r = [ps1.tile([128, 2048], BF16, name=f"abr{b_}")
                        for b_ in range(BLOC)]
                ab_i = [ps1.tile([128, 2048], BF16, name=f"abi{b_}")
                        for b_ in range(BLOC)]
                with tc.tile_pool(name="pI1", bufs=3, space="PSUM") as pi1, \
                        tc.tile_pool(name="pI1s", bufs=3) as pi1s:
                    for g in range(8):
                        for s in range(8):
                            b_, r = divmod(g * 8 + s, R32)
                            o1 = pi1.tile([128, 128], F32, tag="o1")
                            o2 = pi1.tile([128, 128], F32, tag="o2")
                            yts = yt_tiles[g][:, s * 128:(s + 1) * 128]
                            nc.tensor.matmul(o1[:], g1_s[:], yts,
                                             start=True, stop=True)
                            nc.tensor.matmul(o2[:], g2_s[:], yts,
                                             start=True, stop=True)
                            o1s = pi1s.tile([128, 128], BF16, tag="o1s")
                            cp(o1s[:], o1[:])
                            nc.vector.tensor_sub(
                                ab_r[b_][:, r * 64:(r + 1) * 64],
                                o1s[:, 0:64], o2[:, 64:128])
                            nc.vector.tensor_add(
                                ab_i[b_][:, r * 64:(r + 1) * 64],
                                o2[:, 0:64], o1s[:, 64:128])

                # ---- As transposes + I2 -> zs (2 e / 4 z per psum tile) ----
                with tc.tile_pool(name="pAS", bufs=2, space="PSUM") as pas, \
                        tc.tile_pool(name="as2p", bufs=2) as pas2, \
                        tc.tile_pool(name="pZ", bufs=2, space="PSUM") as pz:
                    pzt = None
                    for eg in range(32):
                        asp = pas.tile([64, 512], BF16, tag="asp")
                        for ei in range(2):
                            e = eg * 2 + ei
                            for half in range(2):
                                b_, o_ = divmod(e * 2 + half, C)
                                arv = ab_r[b_][:].rearrange(
                                    "p (r o) -> p o r", o=64)
                                aiv = ab_i[b_][:].rearrange(
                                    "p (r o) -> p o r", o=64)
                                coff = ei * 256 + half * 128
                                nc.tensor.matmul(
                                    asp[0:32, coff:coff + 128],
                                    arv[:, o_, :], ident_s[:, :],
                                    is_transpose=True, start=True, stop=True)
                                nc.tensor.matmul(
                                    asp[32:64, coff:coff + 128],
                                    aiv[:, o_, :], ident_s[:, :],
                                    is_transpose=True, start=True, stop=True)
                        as2 = pas2.tile([64, 512], BF16, tag="as2")
                        cp(as2[:], asp[:])
                        for ei in range(2):
                            for half in range(2):
                                mgl = eg * 4 + ei * 2 + half
                                b_, o_ = divmod(mgl, C)
                                slot = mgl % 4
                                if slot == 0:
                                    pzt = pz.tile([128, 512], F32, tag="zp")
                                nc.tensor.matmul(
                                    pzt[:, slot * 128:(slot + 1) * 128],
                                    as2[:, ei * 256 + half * 128:
                                        ei * 256 + half * 128 + 128],
                                    ahi_s[:], start=True, stop=True)
                                if slot == 3:
                                    cp(zs[b_][:, (o_ - 3) * H:(o_ + 1) * H],
                                       pzt[:])

            # ================= conv + z + KAN (software-pipelined) ====
            with tc.tile_pool(name="pk0", bufs=2, space="PSUM") as pk0, \
                    tc.tile_pool(name="pk1", bufs=1, space="PSUM") as pk1, \
                    tc.tile_pool(name="pk2", bufs=1, space="PSUM") as pk2, \
                    tc.tile_pool(name="pab", bufs=2) as pab, \
                    tc.tile_pool(name="pmb", bufs=2) as pmb, \
                    tc.tile_pool(name="pfb", bufs=3) as pfb, \
                    tc.tile_pool(name="psl", bufs=3) as psl, \
                    tc.tile_pool(name="pob", bufs=3) as pob, \
                    tc.tile_pool(name="pxt", bufs=4) as pxt:
                NCHT = BLOC * NCH
                st = [dict() for _ in range(NCHT)]

                def kan_s0(i):
                    b_, hg = divmod(i, NCH)
                    zv = zs[b_][:].rearrange("p (o h) -> p h o", o=64)
                    xt = pxt.tile([65, CHUNK], BF16, tag="xt")
                    nc.gpsimd.memset(xt[64:65, :], 1.0)
                    nc.sync.dma_start(
                        xt[0:64, :].rearrange("c (h w) -> c h w", w=W),
                        x_in.ap().rearrange("(b c) h w -> b c h w", b=BLOC)
                        [b_, :, hg * HROWS:(hg + 1) * HROWS, :])
                    P0 = pk0.tile([128, CHUNK], F32, tag="p0")
                    for n in range(2):
                        nc.tensor.matmul(
                            P0[:, n * 512:(n + 1) * 512], convw_s[:],
                            xt[:, n * 512:(n + 1) * 512],
                            start=True, stop=False)
                    for hh in range(HROWS):
                        h_ = hg * HROWS + hh
                        nc.tensor.matmul(
                            P0[0:64, hh * 128:(hh + 1) * 128],
                            zv[:, h_, :], ident_s[:, :],
                            start=False, stop=True)
                        nc.tensor.matmul(
                            P0[64:128, hh * 128:(hh + 1) * 128],
                            zv[:, h_, :], ident_s[:, :],
                            start=False, stop=True)
                    st[i]['P0'] = P0

                def elemwise(i, Pin, tagsfx):
                    """basis + silu staging from a [128,CHUNK] psum."""
                    ab_t = pab.tile([128, 4 * CHUNK], BF16, tag="ab" + tagsfx)
                    for p in range(4):
                        nc.scalar.activation(
                            ab_t[:, p * CHUNK:(p + 1) * CHUNK], Pin[:, :],
                            AF.Abs, bias=biasv_s[:, p:p + 1], scale=2.5)
                    tb = psl.tile([64, CHUNK], BF16, tag="tb" + tagsfx)
                    nc.scalar.activation(tb[:], Pin[0:64, :], AF.Tanh,
                                         scale=0.5)
                    mb = pmb.tile([128, 4 * CHUNK], BF16, tag="mb" + tagsfx)
                    nc.vector.tensor_scalar(mb[:], ab_t[:], 2.0, 0.0,
                                            ALU.subtract, ALU.min)
                    fb = pfb.tile([128, 4 * CHUNK], BF16, tag="fb" + tagsfx)
                    nc.vector._custom_dve(ops['cube'], out=fb[:], in0=mb[:],
                                          s0=4.0)
                    sl_t = psl.tile([64, CHUNK], BF16, tag="sl" + tagsfx)
                    nc.vector.scalar_tensor_tensor(
                        sl_t[:], tb[:], 1.0, Pin[0:64, :],
                        ALU.add, ALU.mult)
                    return fb, sl_t

                def kan_mm(Pout, rows, wsl, bsl, mwid, fb, sl_t):
                    for n in range(2):
                        sl = slice(n * 512, (n + 1) * 512)
                        for p in range(4):
                            nc.tensor.matmul(
                                Pout[rows, sl],
                                wsl[:, p * mwid:(p + 1) * mwid],
                                fb[:, p * CHUNK + n * 512:
                                   p * CHUNK + (n + 1) * 512],
                                start=(p == 0), stop=False)
                        nc.tensor.matmul(Pout[rows, sl], bsl[:],
                                         sl_t[:, sl], start=False, stop=True)

                def kan_s1(i):
                    st[i]['fb1'], st[i]['sl1'] = \
                        elemwise(i, st[i]['P0'], "1")

                def kan_s2(i):
                    P1 = pk1.tile([128, CHUNK], F32, tag="p1")
                    kan_mm(P1, slice(0, 128), k1w_s, k1b_s, 128,
                           st[i]['fb1'], st[i]['sl1'])
                    st[i]['fb2'], st[i]['sl2'] = elemwise(i, P1, "2")

                def kan_s3(i):
                    b_, hg = divmod(i, NCH)
                    P2 = pk2.tile([128, CHUNK], F32, tag="p2")
                    kan_mm(P2, slice(0, 64), k2w_s, k2b_s, 64,
                           st[i]['fb2'], st[i]['sl2'])
                    ob = pob.tile([64, CHUNK], BF16, tag="ob")
                    nc.scalar.activation(ob[:], P2[0:64, :], AF.Gelu)
                    nc.sync.dma_start(
                        out.ap()[b_ * C:(b_ + 1) * C,
                                 hg * CHUNK:(hg + 1) * CHUNK], ob[:])
                    st[i].clear()

                for i in range(NCHT + 3):
                    if i < NCHT:
                        kan_s0(i)
                    if 0 <= i - 1 < NCHT:
                        kan_s1(i - 1)
                    if 0 <= i - 2 < NCHT:
                        kan_s2(i - 2)
                    if 0 <= i - 3 < NCHT:
                        kan_s3(i - 3)
    nc.compile()
    return nc


# ---------------------------------------------------------------- runner
_NC = None


def _get_nc():
    global _NC
    if _NC is None:
        _NC = build_nc()
    return _NC


def kernel(x, spec_w1_r, spec_w1_i, spec_w2_r, spec_w2_i, conv_w, conv_b,
           k1_base, k1_spline, k1_scaler, k2_base, k2_spline, k2_scaler):
    nc = _get_nc()
    in_maps = _host_prep(x, spec_w1_r, spec_w1_i, spec_w2_r, spec_w2_i,
                         conv_w, conv_b, k1_base, k1_spline, k1_scaler,
                         k2_base, k2_spline, k2_scaler)
    res = bass_utils.run_bass_kernel_spmd(nc, in_maps,
                                          core_ids=list(range(NCORES)))
    outs = [res.results[c]['out'] for c in range(NCORES)]
    y = np.stack(outs).astype(np.float32)
    return y.reshape(B, C, H, W)
